# revision 1
# baseline (speedup 1.0000x reference)
"""Trainium2 Bass kernel for LoopConnectivityDecoder.

Math: out[i,j] (i<j) = sigmoid( sum_k W2[k] * relu(a'[i,k] + b'[k,j]) + b2 ),
symmetrized, zero diagonal; a' = X@W1[:,:32].T + b1, b' = (X@W1[:,32:].T).T.

Device strategy (8 cores, SPMD, per-core work fixed by host-side gathers):
- Signed scale folded into data: z_k = W2[k]*a' + W2[k]*b'. Then
  W2[k]*relu(a'+b') = max(z,0) if W2[k]>=0 else min(z,0).
- Upper triangle covered by 24 uniform (128 x 512) units, 3 per core.
- Per k: one K=4 bf16 matmul computes the outer sum z in PSUM at full fp32
  accuracy via hi/lo bf16 splitting: lhsT=[a_hi;a_lo;1;1], rhs=[1;1;b_hi;b_lo].
- k's are sign-grouped and chunked by 4 (groups zero-padded to 4-multiples):
  4 matmuls fill a (128,4,512) PSUM tile; ScalarE drains it with one fused
  relu (scale=+/-1 by sign) into SBUF; VectorE/GpSimd run 4-wide interleaved
  accumulate chains (scalar_tensor_tensor: acc = staged*(+/-1) + acc).
- Tail per unit: merge chains, sigmoid(+b2) on ScalarE, DMA out.
- Host scatters unit tiles into the full matrix, applies triu, mirrors.
"""

import numpy as np
import ml_dtypes

N = 1536
EMB = 32
H = 64
P = 128          # partition tile (rows per unit)
F = 512          # free-dim tile (cols per unit)
NCORES = 8
NBLK = N // P    # 12 row blocks
UNITS_PER_CORE = 3
CH = 4           # k's per chunk (PSUM tile = CH banks; build-time override)
LDG = 8          # k-slots per DMA load group

_cache = {}


def _unit_list():
    """24 (row_block, col0) units covering the upper-triangle staircase."""
    units = []
    for bi in range(NBLK):
        cols = N - P * bi
        nch = -(-cols // F)
        for t in range(nch):
            col0 = min(P * bi + F * t, N - F)
            units.append((bi, col0))
    assert len(units) == NCORES * UNITS_PER_CORE
    return units


def _slot_list(pos_mask, ch=CH):
    """Sign-grouped, zero-padded slot list.

    Returns (slots, chunk_signs): slots[i] is a k index or None (zero pad);
    chunk_signs[c] is +1/-1 for slots[ch*c : ch*(c+1)]."""
    pos = [k for k in range(H) if pos_mask[k]]
    neg = [k for k in range(H) if not pos_mask[k]]
    slots, signs = [], []
    for grp, sgn in ((pos, 1.0), (neg, -1.0)):
        if not grp:
            continue
        pad = (-len(grp)) % ch
        g = [None] * pad + grp
        slots += g
        signs += [sgn] * (len(g) // ch)
    assert len(slots) % ch == 0
    return slots, signs


def _build_module(pos_mask, repeat=1, n_dve_chunks=None, ablate="full",
                  stg_bufs=4, psum_bufs=2, stage_bf16=False, ch=CH):
    """Build + compile the Bass module. pos_mask: tuple of 64 bools."""
    from contextlib import ExitStack
    import concourse.tile as tile
    from concourse import bacc, mybir

    slots, signs = _slot_list(pos_mask, ch)
    S = len(slots)
    NCH = S // ch
    NLD = -(-S // LDG)
    if n_dve_chunks is None:
        n_dve_chunks = max(1, min(NCH - 1, round(NCH * 11 / 17)))
    if ablate == "nopool":
        n_dve_chunks = NCH

    nc = bacc.Bacc("TRN2", target_bir_lowering=False, debug=False,
                   num_devices=NCORES)
    A1_d = nc.dram_tensor("A1g", [4, S, UNITS_PER_CORE * P], mybir.dt.bfloat16,
                          kind="ExternalInput")
    B1_d = nc.dram_tensor("B1g", [4, S, UNITS_PER_CORE * F], mybir.dt.bfloat16,
                          kind="ExternalInput")
    b2_d = nc.dram_tensor("b2c", [P, 1], mybir.dt.float32, kind="ExternalInput")
    out_d = nc.dram_tensor("out", [UNITS_PER_CORE, P, F], mybir.dt.float32,
                           kind="ExternalOutput")

    with tile.TileContext(nc) as tc, ExitStack() as ctx:
        const = ctx.enter_context(tc.tile_pool(name="const", bufs=1))
        ld = ctx.enter_context(tc.tile_pool(name="ld", bufs=4))
        stg = ctx.enter_context(tc.tile_pool(name="stg", bufs=stg_bufs))
        accp = ctx.enter_context(tc.tile_pool(name="accp", bufs=2))
        outp = ctx.enter_context(tc.tile_pool(name="outp", bufs=2))
        psum = ctx.enter_context(tc.tile_pool(name="psum", bufs=psum_bufs, space="PSUM"))

        b2_t = const.tile([P, 1], mybir.dt.float32)
        nc.sync.dma_start(b2_t[:], b2_d[:])

        def body():
            for u in range(UNITS_PER_CORE):
                a_tiles, b_tiles = [], []
                for g in range(NLD):
                    s0 = g * LDG
                    sw = min(LDG, S - s0)
                    a_t = ld.tile([4, LDG, P], mybir.dt.bfloat16, tag="a")
                    nc.sync.dma_start(
                        a_t[:, 0:sw], A1_d[:, s0:s0 + sw, u * P:(u + 1) * P])
                    b_t = ld.tile([4, LDG, F], mybir.dt.bfloat16, tag="b")
                    nc.sync.dma_start(
                        b_t[:, 0:sw], B1_d[:, s0:s0 + sw, u * F:(u + 1) * F])
                    a_tiles.append(a_t)
                    b_tiles.append(b_t)

                accD = accN = None
                for c in range(NCH):
                    sgn = signs[c]
                    y = psum.tile([P, ch, F], mybir.dt.float32, tag="y")
                    for q in range(ch):
                        s = c * ch + q
                        g, off = s // LDG, s % LDG
                        nc.tensor.matmul(y[:, q],
                                         a_tiles[g][0:4, off, :],
                                         b_tiles[g][0:4, off, :],
                                         start=True, stop=True)
                    sdt = mybir.dt.bfloat16 if stage_bf16 else mybir.dt.float32
                    t4 = stg.tile([P, ch, F], sdt, tag="t4")
                    nc.scalar.activation(t4[:], y[:],
                                         mybir.ActivationFunctionType.Relu,
                                         scale=float(sgn))
                    if ablate == "noacc":
                        if c == NCH - 1:
                            accD = t4
                        continue
                    # accumulate: acc += sgn * t4 (4-wide interleaved chain)
                    on_dve = c < n_dve_chunks
                    if on_dve:
                        newacc = accp.tile([P, ch, F], mybir.dt.float32,
                                           tag="accD")
                        if accD is None:
                            nc.vector.tensor_scalar(newacc[:], t4[:],
                                                    float(sgn), None,
                                                    mybir.AluOpType.mult)
                        else:
                            nc.vector.scalar_tensor_tensor(
                                newacc[:], t4[:], float(sgn), accD[:],
                                mybir.AluOpType.mult, mybir.AluOpType.add)
                        accD = newacc
                    else:
                        # gpsimd: walrus rejects TensorScalarPtr on Pool, so
                        # chain with plain tensor_tensor add/subtract.
                        newacc = accp.tile([P, ch, F], mybir.dt.float32,
                                           tag="accN")
                        if accN is None:
                            accN = accp.tile([P, ch, F], mybir.dt.float32,
                                             tag="accN")
                            nc.gpsimd.memset(accN[:], 0.0)
                        op = (mybir.AluOpType.add if sgn > 0
                              else mybir.AluOpType.subtract)
                        nc.gpsimd.tensor_tensor(newacc[:], accN[:], t4[:], op)
                        accN = newacc

                # merge chains: logit = sum over ch slices (+ gpsimd chain)
                lg = outp.tile([P, F], mybir.dt.float32, tag="lg")
                def fold(eng, acc):
                    w = ch
                    while w > 1:
                        half = w // 2
                        nxt = outp.tile([P, half, F], mybir.dt.float32,
                                        tag=f"fold{half}")
                        eng.tensor_tensor(nxt[:], acc[:, 0:half],
                                          acc[:, half:2 * half],
                                          mybir.AluOpType.add)
                        acc, w = nxt, half
                    return acc
                aD = fold(nc.vector, accD)
                if accN is not None and ablate != "noacc":
                    aN = fold(nc.gpsimd, accN)
                    nc.vector.tensor_tensor(lg[:], aD[:, 0], aN[:, 0],
                                            mybir.AluOpType.add)
                else:
                    nc.vector.tensor_copy(lg[:], aD[:, 0])
                s_t = outp.tile([P, F], mybir.dt.float32, tag="s")
                nc.scalar.activation(s_t[:], lg[:],
                                     mybir.ActivationFunctionType.Sigmoid,
                                     bias=b2_t[:, 0:1], scale=1.0)
                nc.sync.dma_start(out_d[u], s_t[:])

        if repeat > 1:
            with tc.For_i(0, repeat, 1):
                body()
        else:
            body()

    nc.compile()
    return nc


def _split_bf16(x):
    """Split fp32 array into (hi, lo) bf16 arrays with hi+lo ~= x."""
    hi = x.astype(ml_dtypes.bfloat16)
    lo = (x - hi.astype(np.float32)).astype(ml_dtypes.bfloat16)
    return hi, lo


def _prep_inputs(loop_embeddings, W1, b1, W2, b2):
    X = np.asarray(loop_embeddings, dtype=np.float32)
    W1 = np.asarray(W1, dtype=np.float32)
    b1 = np.asarray(b1, dtype=np.float32)
    W2 = np.asarray(W2, dtype=np.float32)
    b2 = np.asarray(b2, dtype=np.float32)

    a = X @ W1[:, :EMB].T + b1          # (N, H)
    bm = X @ W1[:, EMB:].T              # (N, H)
    w2 = W2[0]

    az = (w2[None, :] * a).T            # (H, N): z-contribution rows (i)
    bz = (w2[None, :] * bm).T           # (H, N): z-contribution rows (j)
    az_hi, az_lo = _split_bf16(az)
    bz_hi, bz_lo = _split_bf16(bz)

    pos_mask = tuple(bool(v) for v in (w2 >= 0))
    slots, _ = _slot_list(pos_mask)
    S = len(slots)
    units = _unit_list()

    in_maps = []
    for core in range(NCORES):
        A1g = np.zeros((4, S, UNITS_PER_CORE * P), dtype=ml_dtypes.bfloat16)
        B1g = np.zeros((4, S, UNITS_PER_CORE * F), dtype=ml_dtypes.bfloat16)
        for u in range(UNITS_PER_CORE):
            bi, col0 = units[core * UNITS_PER_CORE + u]
            for s, k in enumerate(slots):
                if k is None:
                    continue
                A1g[0, s, u * P:(u + 1) * P] = az_hi[k, bi * P:(bi + 1) * P]
                A1g[1, s, u * P:(u + 1) * P] = az_lo[k, bi * P:(bi + 1) * P]
                A1g[2, s, u * P:(u + 1) * P] = 1.0
                A1g[3, s, u * P:(u + 1) * P] = 1.0
                B1g[0, s, u * F:(u + 1) * F] = 1.0
                B1g[1, s, u * F:(u + 1) * F] = 1.0
                B1g[2, s, u * F:(u + 1) * F] = bz_hi[k, col0:col0 + F]
                B1g[3, s, u * F:(u + 1) * F] = bz_lo[k, col0:col0 + F]
        in_maps.append({
            "A1g": A1g,
            "B1g": B1g,
            "b2c": np.full((P, 1), b2[0], dtype=np.float32),
        })
    return in_maps, pos_mask, units


def kernel(loop_embeddings, W1, b1, W2, b2):
    from concourse.bass_utils import run_bass_kernel_spmd

    in_maps, pos_mask, units = _prep_inputs(loop_embeddings, W1, b1, W2, b2)

    if pos_mask not in _cache:
        _cache[pos_mask] = _build_module(pos_mask)
    nc = _cache[pos_mask]

    res = run_bass_kernel_spmd(nc, in_maps, list(range(NCORES)))

    s = np.zeros((N, N), dtype=np.float32)
    for core in range(NCORES):
        o = res.results[core]["out"]
        for u in range(UNITS_PER_CORE):
            bi, col0 = units[core * UNITS_PER_CORE + u]
            s[bi * P:(bi + 1) * P, col0:col0 + F] = o[u]
    up = np.triu(s, 1)
    return (up + up.T).astype(np.float32)



# revision 4
# speedup vs baseline: 1.4415x; 1.4415x over previous
"""Trainium2 Bass kernel for LoopConnectivityDecoder.

Math: out[i,j] (i<j) = sigmoid( sum_k W2[k] * relu(a'[i,k] + b'[k,j]) + b2 ),
symmetrized, zero diagonal; a' = X@W1[:,:32].T + b1, b' = (X@W1[:,32:].T).T.

Device strategy (8 cores, SPMD, per-core work fixed by host-side gathers):
- Signed scale folded into data: z_k = W2[k]*a' + W2[k]*b'. Then
  W2[k]*relu(a'+b') = max(z,0) if W2[k]>=0 else min(z,0).
- Upper triangle covered by 24 uniform (128 x 512) units, 3 per core.
- Per k: one K=4 bf16 matmul computes the outer sum z in PSUM at full fp32
  accuracy via hi/lo bf16 splitting: lhsT=[a_hi;a_lo;1;1], rhs=[1;1;b_hi;b_lo].
- k's are sign-grouped and chunked by 4 (groups zero-padded to 4-multiples):
  4 matmuls fill a (128,4,512) PSUM tile; ScalarE drains it with one fused
  relu (scale=+/-1 by sign) into SBUF; VectorE/GpSimd run 4-wide interleaved
  accumulate chains (scalar_tensor_tensor: acc = staged*(+/-1) + acc).
- Tail per unit: merge chains, sigmoid(+b2) on ScalarE, DMA out.
- Host scatters unit tiles into the full matrix, applies triu, mirrors.

Dispatch strategy: the axon tunnel has ~80ms round-trip latency and
~50-70 MB/s bandwidth, so the executable is traced/compiled ONCE and kept
module-level (run_bass_kernel_spmd re-jits a fresh closure per call, which
re-serializes the whole BIR into the HLO each time). Per-call work is just
host prep (vectorized), one jitted exec, one fetch, and host assembly.
"""

import numpy as np
import ml_dtypes

N = 1536
EMB = 32
H = 64
P = 128          # partition tile (rows per unit)
F = 512          # free-dim tile (cols per unit)
NCORES = 8
NBLK = N // P    # 12 row blocks
UNITS_PER_CORE = 3
CH = 4           # k's per chunk (PSUM tile = CH banks; build-time override)
LDG = 8          # k-slots per DMA load group

_cache = {}


def _unit_list():
    """24 (row_block, col0) units covering the upper-triangle staircase."""
    units = []
    for bi in range(NBLK):
        cols = N - P * bi
        nch = -(-cols // F)
        for t in range(nch):
            col0 = min(P * bi + F * t, N - F)
            units.append((bi, col0))
    assert len(units) == NCORES * UNITS_PER_CORE
    return units


def _slot_list(pos_mask, ch=CH):
    """Sign-grouped, zero-padded slot list.

    Returns (slots, chunk_signs): slots[i] is a k index or None (zero pad);
    chunk_signs[c] is +1/-1 for slots[ch*c : ch*(c+1)]."""
    pos = [k for k in range(H) if pos_mask[k]]
    neg = [k for k in range(H) if not pos_mask[k]]
    slots, signs = [], []
    for grp, sgn in ((pos, 1.0), (neg, -1.0)):
        if not grp:
            continue
        pad = (-len(grp)) % ch
        g = [None] * pad + grp
        slots += g
        signs += [sgn] * (len(g) // ch)
    assert len(slots) % ch == 0
    return slots, signs


def _build_module(pos_mask, repeat=1, n_dve_chunks=None, ablate="full",
                  stg_bufs=4, psum_bufs=2, stage_bf16=False, ch=CH):
    """Build + compile the Bass module. pos_mask: tuple of 64 bools."""
    from contextlib import ExitStack
    import concourse.tile as tile
    from concourse import bacc, mybir

    slots, signs = _slot_list(pos_mask, ch)
    S = len(slots)
    NCH = S // ch
    NLD = -(-S // LDG)
    if n_dve_chunks is None:
        n_dve_chunks = max(1, min(NCH - 1, round(NCH * 11 / 17)))
    if ablate == "nopool":
        n_dve_chunks = NCH

    nc = bacc.Bacc("TRN2", target_bir_lowering=False, debug=False,
                   num_devices=NCORES)
    A1_d = nc.dram_tensor("A1g", [4, S, UNITS_PER_CORE * P], mybir.dt.bfloat16,
                          kind="ExternalInput")
    B1_d = nc.dram_tensor("B1g", [4, S, UNITS_PER_CORE * F], mybir.dt.bfloat16,
                          kind="ExternalInput")
    b2_d = nc.dram_tensor("b2c", [P, 1], mybir.dt.float32, kind="ExternalInput")
    out_d = nc.dram_tensor("out", [UNITS_PER_CORE, P, F], mybir.dt.float32,
                           kind="ExternalOutput")

    with tile.TileContext(nc) as tc, ExitStack() as ctx:
        const = ctx.enter_context(tc.tile_pool(name="const", bufs=1))
        ld = ctx.enter_context(tc.tile_pool(name="ld", bufs=4))
        stg = ctx.enter_context(tc.tile_pool(name="stg", bufs=stg_bufs))
        accp = ctx.enter_context(tc.tile_pool(name="accp", bufs=2))
        outp = ctx.enter_context(tc.tile_pool(name="outp", bufs=2))
        psum = ctx.enter_context(tc.tile_pool(name="psum", bufs=psum_bufs, space="PSUM"))

        b2_t = const.tile([P, 1], mybir.dt.float32)
        nc.sync.dma_start(b2_t[:], b2_d[:])

        def body():
            for u in range(UNITS_PER_CORE):
                a_tiles, b_tiles = [], []
                for g in range(NLD):
                    s0 = g * LDG
                    sw = min(LDG, S - s0)
                    a_t = ld.tile([4, LDG, P], mybir.dt.bfloat16, tag="a")
                    nc.sync.dma_start(
                        a_t[:, 0:sw], A1_d[:, s0:s0 + sw, u * P:(u + 1) * P])
                    b_t = ld.tile([4, LDG, F], mybir.dt.bfloat16, tag="b")
                    nc.sync.dma_start(
                        b_t[:, 0:sw], B1_d[:, s0:s0 + sw, u * F:(u + 1) * F])
                    a_tiles.append(a_t)
                    b_tiles.append(b_t)

                accD = accN = None
                for c in range(NCH):
                    sgn = signs[c]
                    y = psum.tile([P, ch, F], mybir.dt.float32, tag="y")
                    for q in range(ch):
                        s = c * ch + q
                        g, off = s // LDG, s % LDG
                        nc.tensor.matmul(y[:, q],
                                         a_tiles[g][0:4, off, :],
                                         b_tiles[g][0:4, off, :],
                                         start=True, stop=True)
                    sdt = mybir.dt.bfloat16 if stage_bf16 else mybir.dt.float32
                    t4 = stg.tile([P, ch, F], sdt, tag="t4")
                    nc.scalar.activation(t4[:], y[:],
                                         mybir.ActivationFunctionType.Relu,
                                         scale=float(sgn))
                    if ablate == "noacc":
                        if c == NCH - 1:
                            accD = t4
                        continue
                    # accumulate: acc += sgn * t4 (4-wide interleaved chain)
                    on_dve = c < n_dve_chunks
                    if on_dve:
                        newacc = accp.tile([P, ch, F], mybir.dt.float32,
                                           tag="accD")
                        if accD is None:
                            nc.vector.tensor_scalar(newacc[:], t4[:],
                                                    float(sgn), None,
                                                    mybir.AluOpType.mult)
                        else:
                            nc.vector.scalar_tensor_tensor(
                                newacc[:], t4[:], float(sgn), accD[:],
                                mybir.AluOpType.mult, mybir.AluOpType.add)
                        accD = newacc
                    else:
                        # gpsimd: walrus rejects TensorScalarPtr on Pool, so
                        # chain with plain tensor_tensor add/subtract.
                        newacc = accp.tile([P, ch, F], mybir.dt.float32,
                                           tag="accN")
                        if accN is None:
                            accN = accp.tile([P, ch, F], mybir.dt.float32,
                                             tag="accN")
                            nc.gpsimd.memset(accN[:], 0.0)
                        op = (mybir.AluOpType.add if sgn > 0
                              else mybir.AluOpType.subtract)
                        nc.gpsimd.tensor_tensor(newacc[:], accN[:], t4[:], op)
                        accN = newacc

                # merge chains: logit = sum over ch slices (+ gpsimd chain)
                lg = outp.tile([P, F], mybir.dt.float32, tag="lg")
                def fold(eng, acc):
                    w = ch
                    while w > 1:
                        half = w // 2
                        nxt = outp.tile([P, half, F], mybir.dt.float32,
                                        tag=f"fold{half}")
                        eng.tensor_tensor(nxt[:], acc[:, 0:half],
                                          acc[:, half:2 * half],
                                          mybir.AluOpType.add)
                        acc, w = nxt, half
                    return acc
                aD = fold(nc.vector, accD)
                if accN is not None and ablate != "noacc":
                    aN = fold(nc.gpsimd, accN)
                    nc.vector.tensor_tensor(lg[:], aD[:, 0], aN[:, 0],
                                            mybir.AluOpType.add)
                else:
                    nc.vector.tensor_copy(lg[:], aD[:, 0])
                s_t = outp.tile([P, F], mybir.dt.float32, tag="s")
                nc.scalar.activation(s_t[:], lg[:],
                                     mybir.ActivationFunctionType.Sigmoid,
                                     bias=b2_t[:, 0:1], scale=1.0)
                nc.sync.dma_start(out_d[u], s_t[:])

        if repeat > 1:
            with tc.For_i(0, repeat, 1):
                body()
        else:
            body()

    nc.compile()
    return nc


def _split_bf16(x):
    """Split fp32 array into (hi, lo) bf16 arrays with hi+lo ~= x."""
    hi = x.astype(ml_dtypes.bfloat16)
    lo = (x - hi.astype(np.float32)).astype(ml_dtypes.bfloat16)
    return hi, lo


def _prep_inputs(loop_embeddings, W1, b1, W2, b2):
    """Vectorized per-core input build.

    Returns (concat arrays keyed by tensor name, pos_mask, units)."""
    X = np.asarray(loop_embeddings, dtype=np.float32)
    W1 = np.asarray(W1, dtype=np.float32)
    b1 = np.asarray(b1, dtype=np.float32)
    W2 = np.asarray(W2, dtype=np.float32)
    b2 = np.asarray(b2, dtype=np.float32)

    a = X @ W1[:, :EMB].T + b1          # (N, H)
    bm = X @ W1[:, EMB:].T              # (N, H)
    w2 = W2[0]

    az = (w2[None, :] * a).T            # (H, N): z-contribution rows (i)
    bz = (w2[None, :] * bm).T           # (H, N): z-contribution rows (j)
    az_hi, az_lo = _split_bf16(az)
    bz_hi, bz_lo = _split_bf16(bz)

    pos_mask = tuple(bool(v) for v in (w2 >= 0))
    slots, _ = _slot_list(pos_mask)
    S = len(slots)
    units = _unit_list()

    # static gather indices (cached on pos_mask irrelevant; cheap to build)
    kmap = np.array([0 if k is None else k for k in slots], dtype=np.int64)
    kvalid = np.array([k is not None for k in slots], dtype=bool)
    # per-core column index arrays
    acols = np.empty((NCORES, UNITS_PER_CORE * P), dtype=np.int64)
    bcols = np.empty((NCORES, UNITS_PER_CORE * F), dtype=np.int64)
    for core in range(NCORES):
        for u in range(UNITS_PER_CORE):
            bi, col0 = units[core * UNITS_PER_CORE + u]
            acols[core, u * P:(u + 1) * P] = np.arange(bi * P, (bi + 1) * P)
            bcols[core, u * F:(u + 1) * F] = np.arange(col0, col0 + F)

    # hoisted slot-gathers (S, N), invalid slots zeroed
    azh, azl = az_hi[kmap], az_lo[kmap]
    bzh, bzl = bz_hi[kmap], bz_lo[kmap]
    for arr in (azh, azl, bzh, bzl):
        arr[~kvalid] = 0
    ones_col = kvalid.astype(ml_dtypes.bfloat16)[:, None]   # (S, 1)

    # concatenated-core layout: (NCORES*4, S, cols)
    A1all = np.zeros((NCORES, 4, S, UNITS_PER_CORE * P), dtype=ml_dtypes.bfloat16)
    B1all = np.zeros((NCORES, 4, S, UNITS_PER_CORE * F), dtype=ml_dtypes.bfloat16)
    for core in range(NCORES):
        ac, bc = acols[core], bcols[core]
        A1all[core, 0] = azh[:, ac]
        A1all[core, 1] = azl[:, ac]
        A1all[core, 2] = ones_col
        A1all[core, 3] = ones_col
        B1all[core, 2] = bzh[:, bc]
        B1all[core, 3] = bzl[:, bc]
        B1all[core, 0] = ones_col
        B1all[core, 1] = ones_col
    concat = {
        "A1g": A1all.reshape(NCORES * 4, S, UNITS_PER_CORE * P),
        "B1g": B1all.reshape(NCORES * 4, S, UNITS_PER_CORE * F),
        "b2c": np.tile(np.full((P, 1), b2[0], dtype=np.float32), (NCORES, 1)),
    }
    return concat, pos_mask, units


class _Executor:
    """Persistent jitted shard_map executable for a compiled Bass module."""

    def __init__(self, nc):
        import jax
        from jax.sharding import Mesh, PartitionSpec, NamedSharding
        from jax.experimental.shard_map import shard_map
        from concourse import mybir
        from concourse.bass2jax import (_bass_exec_p, install_neuronx_cc_hook,
                                        partition_id_tensor)

        install_neuronx_cc_hook()
        self.jax = jax
        self.nc = nc
        partition_name = (nc.partition_id_tensor.name
                          if nc.partition_id_tensor else None)
        in_names, out_names, out_avals, zero_outs = [], [], [], []
        for alloc in nc.m.functions[0].allocations:
            if not isinstance(alloc, mybir.MemoryLocationSet):
                continue
            name = alloc.memorylocations[0].name
            if alloc.kind == "ExternalInput":
                if name != partition_name:
                    in_names.append(name)
            elif alloc.kind == "ExternalOutput":
                out_names.append(name)
                shape = tuple(alloc.tensor_shape)
                dtype = mybir.dt.np(alloc.dtype)
                out_avals.append(jax.core.ShapedArray(shape, dtype))
                zero_outs.append(np.zeros(shape, dtype))
        self.in_names = in_names
        self.out_names = out_names
        self.out_avals = out_avals
        n_params = len(in_names)
        n_outs = len(out_avals)
        in_names_full = list(in_names) + out_names
        if partition_name is not None:
            in_names_full.append(partition_name)

        devices = jax.devices()[:NCORES]
        mesh = Mesh(np.asarray(devices), ("core",))
        self.sharding = NamedSharding(mesh, PartitionSpec("core"))

        def _body(*args):
            operands = list(args)
            if partition_name is not None:
                operands.append(partition_id_tensor())
            outs = _bass_exec_p.bind(
                *operands,
                out_avals=tuple(out_avals),
                in_names=tuple(in_names_full),
                out_names=tuple(out_names),
                lowering_input_output_aliases=(),
                sim_require_finite=True,
                sim_require_nnan=True,
                nc=nc,
            )
            return tuple(outs)

        in_specs = (PartitionSpec("core"),) * (n_params + n_outs)
        out_specs = (PartitionSpec("core"),) * n_outs
        # No donation: the kernel writes every output element, so the
        # pre-zeroed output operands can live on-device permanently.
        self.fn = jax.jit(
            shard_map(_body, mesh=mesh, in_specs=in_specs,
                      out_specs=out_specs, check_rep=False),
            keep_unused=True)
        self.dz = [jax.device_put(
            np.zeros((NCORES * z.shape[0], *z.shape[1:]), z.dtype),
            self.sharding) for z in zero_outs]

    def run(self, concat_map):
        args = [concat_map[name] for name in self.in_names]
        out = self.fn(*args, *self.dz)
        return [np.asarray(o) for o in out]


_TRIU_MASK_P = None


def _assemble(o, units):
    """o: (NCORES*UNITS, P, F) fp32 tiles -> full (N, N) symmetrized."""
    global _TRIU_MASK_P
    if _TRIU_MASK_P is None:
        _TRIU_MASK_P = np.triu(np.ones((P, P), dtype=bool), k=1)
    out = np.zeros((N, N), dtype=np.float32)
    for idx, (bi, col0) in enumerate(units):
        r0 = bi * P
        tile = o[idx]
        c_lo, c_hi = col0, col0 + F
        if c_lo <= r0 < c_hi:
            # diagonal block inside this tile: keep strictly-upper, mirror;
            # cols < r0 are below-diagonal (wrong-side values) -> skip.
            d0 = r0 - c_lo
            dblk = tile[:, d0:d0 + P] * _TRIU_MASK_P
            out[r0:r0 + P, r0:r0 + P] = dblk
            out[r0:r0 + P, r0:r0 + P] += dblk.T
            if d0 + P < F:
                post = tile[:, d0 + P:]
                out[r0:r0 + P, r0 + P:c_hi] = post
                out[r0 + P:c_hi, r0:r0 + P] = post.T
        else:
            out[r0:r0 + P, c_lo:c_hi] = tile
            out[c_lo:c_hi, r0:r0 + P] = tile.T
    return out


def kernel(loop_embeddings, W1, b1, W2, b2):
    concat, pos_mask, units = _prep_inputs(loop_embeddings, W1, b1, W2, b2)

    if pos_mask not in _cache:
        nc = _build_module(pos_mask)
        _cache[pos_mask] = _Executor(nc)
    ex = _cache[pos_mask]

    outs = ex.run(concat)
    o = outs[0].reshape(NCORES * UNITS_PER_CORE, P, F)
    return _assemble(o, units)


# revision 8
# speedup vs baseline: 4.4443x; 3.0832x over previous
"""Trainium2 Bass kernel for LoopConnectivityDecoder.

Math: out[i,j] (i<j) = sigmoid( sum_k W2[k] * relu(a'[i,k] + b'[k,j]) + b2 ),
symmetrized, zero diagonal; a' = X@W1[:,:32].T + b1, b' = (X@W1[:,32:].T).T.

The axon tunnel dominates cost (~70-80ms fixed round-trip, ~50-70MB/s), so
the design minimizes per-call host<->device traffic and keeps a persistent
jitted executable:

- Inputs per core are raw-ish and tiny (~125KB fp16): X.T column-gathers for
  the core's units (XA/XB), slot-permuted w2-scaled W1 halves (Wa/Wb), biases.
- The device computes az[s,i] = w2_s*(X@W1a.T + b1)[i,s] and
  bz[s,j] = w2_s*(X@W1b.T)[j,s] itself with K=32 matmuls into PSUM, drained
  to fp16 SBUF (slot-permuted so sign groups are contiguous).
- Upper triangle covered by 24 uniform (128 x 512) units, 3 per core. Per
  k-slot, z[i,j] = az[s,i] + bz[s,j] via two K=1 PE matmuls accumulating in
  PSUM: (az_row)^T @ ones + ones^T @ bz_row.
- k's are sign-grouped, chunked by 4: ScalarE drains each (128,4,512) PSUM
  tile with fused relu (scale=+/-1), VectorE runs the signed accumulate
  chain, then sigmoid(+b2) and a uint8 quantization (x255) per unit tile.
- Output is uint8 (sigmoid in [0,1]; quantization error ~0.2% << 2e-2
  tolerance), 196KB/core. Host dequantizes + scatters + mirrors.
"""

import numpy as np

N = 1536
EMB = 32
H = 64
P = 128          # partition tile (rows per unit)
F = 512          # free-dim tile (cols per unit)
NCORES = 8
NBLK = N // P    # 12 row blocks
UNITS_PER_CORE = 3
CH = 4           # k's per chunk (PSUM tile = CH banks)

_cache = {}


def _unit_list():
    """24 (row_block, col0) units covering the upper-triangle staircase."""
    units = []
    for bi in range(NBLK):
        cols = N - P * bi
        nch = -(-cols // F)
        for t in range(nch):
            col0 = min(P * bi + F * t, N - F)
            units.append((bi, col0))
    assert len(units) == NCORES * UNITS_PER_CORE
    return units


def _slot_list(pos_mask, ch=CH):
    """Sign-grouped, zero-padded slot list.

    Returns (slots, chunk_signs): slots[i] is a k index or None (zero pad);
    chunk_signs[c] is +1/-1 for slots[ch*c : ch*(c+1)]."""
    pos = [k for k in range(H) if pos_mask[k]]
    neg = [k for k in range(H) if not pos_mask[k]]
    slots, signs = [], []
    for grp, sgn in ((pos, 1.0), (neg, -1.0)):
        if not grp:
            continue
        pad = (-len(grp)) % ch
        g = [None] * pad + grp
        slots += g
        signs += [sgn] * (len(g) // ch)
    assert len(slots) % ch == 0
    return slots, signs


def _static_maps():
    """Cached static gather indices for the unit layout."""
    units = _unit_list()
    acols = np.empty((NCORES, UNITS_PER_CORE * P), dtype=np.int64)
    bcols = np.empty((NCORES, UNITS_PER_CORE * F), dtype=np.int64)
    for core in range(NCORES):
        for u in range(UNITS_PER_CORE):
            bi, col0 = units[core * UNITS_PER_CORE + u]
            acols[core, u * P:(u + 1) * P] = np.arange(bi * P, (bi + 1) * P)
            bcols[core, u * F:(u + 1) * F] = np.arange(col0, col0 + F)
    return units, acols, bcols


_UNITS, _ACOLS, _BCOLS = _static_maps()
_TRIU_MASK_P = np.triu(np.ones((P, P), dtype=bool), k=1)
_DEQ_LUT = (np.arange(256, dtype=np.float32) / 255.0)


def _build_module(pos_mask, ch=CH):
    """Build + compile the Bass module. pos_mask: tuple of 64 bools."""
    from contextlib import ExitStack
    import concourse.tile as tile
    from concourse import bacc, mybir

    slots, signs = _slot_list(pos_mask, ch)
    S = len(slots)
    NCH = S // ch
    f16 = mybir.dt.float16
    f32 = mybir.dt.float32

    nc = bacc.Bacc("TRN2", target_bir_lowering=False, debug=False,
                   num_devices=NCORES)
    XA_d = nc.dram_tensor("XAg", [EMB, UNITS_PER_CORE * P], f16,
                          kind="ExternalInput")
    XB_d = nc.dram_tensor("XBg", [EMB, UNITS_PER_CORE * F], f16,
                          kind="ExternalInput")
    Wa_d = nc.dram_tensor("Wag", [EMB, S], f16, kind="ExternalInput")
    Wb_d = nc.dram_tensor("Wbg", [EMB, S], f16, kind="ExternalInput")
    ba_d = nc.dram_tensor("bag", [S, 1], f32, kind="ExternalInput")
    b2_d = nc.dram_tensor("b2c", [P, 1], f32, kind="ExternalInput")
    out_d = nc.dram_tensor("out", [UNITS_PER_CORE, P, F], mybir.dt.uint8,
                           kind="ExternalOutput")

    with tile.TileContext(nc) as tc, ExitStack() as ctx:
        const = ctx.enter_context(tc.tile_pool(name="const", bufs=1))
        bfp = ctx.enter_context(tc.tile_pool(name="bfp", bufs=1))
        stg = ctx.enter_context(tc.tile_pool(name="stg", bufs=2))
        accp = ctx.enter_context(tc.tile_pool(name="accp", bufs=2))
        outp = ctx.enter_context(tc.tile_pool(name="outp", bufs=2))
        psprep = ctx.enter_context(tc.tile_pool(name="psprep", bufs=1,
                                                space="PSUM"))
        psum = ctx.enter_context(tc.tile_pool(name="psum", bufs=1,
                                              space="PSUM"))

        XA_t = const.tile([EMB, UNITS_PER_CORE * P], f16)
        XB_t = const.tile([EMB, UNITS_PER_CORE * F], f16)
        Wa_t = const.tile([EMB, S], f16)
        Wb_t = const.tile([EMB, S], f16)
        ba_t = const.tile([S, 1], f32)
        b2_t = const.tile([P, 1], f32)
        nc.sync.dma_start(XA_t[:], XA_d[:])
        nc.sync.dma_start(XB_t[:], XB_d[:])
        nc.sync.dma_start(Wa_t[:], Wa_d[:])
        nc.sync.dma_start(Wb_t[:], Wb_d[:])
        nc.sync.dma_start(ba_t[:], ba_d[:])
        nc.sync.dma_start(b2_t[:], b2_d[:])

        onesP = const.tile([1, P], f16)
        onesF = const.tile([1, F], f16)
        nc.vector.memset(onesP[:], 1.0)
        nc.vector.memset(onesF[:], 1.0)

        # on-device operand prep: az/bz for all 3 units, slot-permuted
        psA = psprep.tile([S, UNITS_PER_CORE, P], f32)
        psB = psprep.tile([S, UNITS_PER_CORE, F], f32)
        for u in range(UNITS_PER_CORE):
            nc.tensor.matmul(psA[:, u], Wa_t[:], XA_t[:, u * P:(u + 1) * P],
                             start=True, stop=True)
            nc.tensor.matmul(psB[:, u], Wb_t[:], XB_t[:, u * F:(u + 1) * F],
                             start=True, stop=True)
        azsb = const.tile([S, UNITS_PER_CORE, P], f16)
        bzsb = const.tile([S, UNITS_PER_CORE, F], f16)
        nc.scalar.activation(azsb[:], psA[:],
                             mybir.ActivationFunctionType.Identity,
                             bias=ba_t[:, 0:1], scale=1.0)
        nc.scalar.activation(bzsb[:], psB[:],
                             mybir.ActivationFunctionType.Identity)

        # PE matmul operands must start at partition 0/32/64, so flatten the
        # per-slot rows onto partition 0 (slots along the free dim).
        af = const.tile([1, UNITS_PER_CORE, S, P], f16)
        for u in range(UNITS_PER_CORE):
            nc.sync.dma_start(af[0:1, u], azsb[:, u, :])

        for u in range(UNITS_PER_CORE):
            bf = bfp.tile([1, S, F], f16, tag="bf")
            nc.sync.dma_start(bf[0:1], bzsb[:, u, :])
            accD = None
            for c in range(NCH):
                sgn = signs[c]
                y = psum.tile([P, ch, F], f32, tag="y")
                for q in range(ch):
                    s = c * ch + q
                    nc.tensor.matmul(y[:, q], af[0:1, u, s, :],
                                     onesF[0:1, :], start=True, stop=False)
                    nc.tensor.matmul(y[:, q], onesP[0:1, :],
                                     bf[0:1, s, :],
                                     start=False, stop=True)
                t4 = stg.tile([P, ch, F], f32, tag="t4")
                nc.scalar.activation(t4[:], y[:],
                                     mybir.ActivationFunctionType.Relu,
                                     scale=float(sgn))
                newacc = accp.tile([P, ch, F], f32, tag="accD")
                if accD is None:
                    nc.vector.tensor_scalar(newacc[:], t4[:], float(sgn),
                                            None, mybir.AluOpType.mult)
                else:
                    nc.vector.scalar_tensor_tensor(
                        newacc[:], t4[:], float(sgn), accD[:],
                        mybir.AluOpType.mult, mybir.AluOpType.add)
                accD = newacc

            # fold ch slices -> logit, sigmoid, quantize, store
            acc, w = accD, ch
            while w > 1:
                half = w // 2
                nxt = outp.tile([P, half, F], f32, tag=f"fold{half}")
                nc.vector.tensor_tensor(nxt[:], acc[:, 0:half],
                                        acc[:, half:2 * half],
                                        mybir.AluOpType.add)
                acc, w = nxt, half
            s_t = outp.tile([P, F], f32, tag="s")
            nc.scalar.activation(s_t[:], acc[:, 0],
                                 mybir.ActivationFunctionType.Sigmoid,
                                 bias=b2_t[:, 0:1], scale=1.0)
            q_t = outp.tile([P, F], mybir.dt.uint8, tag="q")
            nc.vector.tensor_scalar(q_t[:], s_t[:], 255.0, None,
                                    mybir.AluOpType.mult)
            nc.sync.dma_start(out_d[u], q_t[:])

    nc.compile()
    return nc


def _prep_inputs(loop_embeddings, W1, b1, W2, b2):
    """Vectorized, tiny per-core input build. Returns (concat dict, pos_mask)."""
    X = np.asarray(loop_embeddings, dtype=np.float32)
    W1 = np.asarray(W1, dtype=np.float32)
    b1 = np.asarray(b1, dtype=np.float32)
    W2 = np.asarray(W2, dtype=np.float32)
    b2 = np.asarray(b2, dtype=np.float32)
    w2 = W2[0]

    pos_mask = tuple(bool(v) for v in (w2 >= 0))
    slots, _ = _slot_list(pos_mask)
    S = len(slots)
    kmap = np.array([0 if k is None else k for k in slots], dtype=np.int64)
    kvalid = np.array([k is not None for k in slots], dtype=bool)

    Wa = (w2[None, :] * W1[:, :EMB].T)[:, kmap].astype(np.float16)
    Wb = (w2[None, :] * W1[:, EMB:].T)[:, kmap].astype(np.float16)
    Wa[:, ~kvalid] = 0
    Wb[:, ~kvalid] = 0
    ba = (w2 * b1)[kmap].astype(np.float32)
    ba[~kvalid] = 0

    XT16 = X.T.astype(np.float16)                      # (EMB, N)
    XA = XT16[:, _ACOLS].transpose(1, 0, 2)            # (NCORES, EMB, 3P)
    XB = XT16[:, _BCOLS].transpose(1, 0, 2)            # (NCORES, EMB, 3F)

    concat = {
        "XAg": np.ascontiguousarray(XA).reshape(NCORES * EMB, -1),
        "XBg": np.ascontiguousarray(XB).reshape(NCORES * EMB, -1),
        "Wag": np.tile(Wa, (NCORES, 1)),
        "Wbg": np.tile(Wb, (NCORES, 1)),
        "bag": np.tile(ba[:, None], (NCORES, 1)),
        "b2c": np.full((NCORES * P, 1), b2[0], dtype=np.float32),
    }
    return concat, pos_mask


class _Executor:
    """Persistent jitted shard_map executable for a compiled Bass module."""

    def __init__(self, nc):
        import jax
        from jax.sharding import Mesh, PartitionSpec, NamedSharding
        from jax.experimental.shard_map import shard_map
        from concourse import mybir
        from concourse.bass2jax import (_bass_exec_p, install_neuronx_cc_hook,
                                        partition_id_tensor)

        install_neuronx_cc_hook()
        self.nc = nc
        partition_name = (nc.partition_id_tensor.name
                          if nc.partition_id_tensor else None)
        in_names, out_names, out_avals, zero_outs = [], [], [], []
        for alloc in nc.m.functions[0].allocations:
            if not isinstance(alloc, mybir.MemoryLocationSet):
                continue
            name = alloc.memorylocations[0].name
            if alloc.kind == "ExternalInput":
                if name != partition_name:
                    in_names.append(name)
            elif alloc.kind == "ExternalOutput":
                out_names.append(name)
                shape = tuple(alloc.tensor_shape)
                dtype = mybir.dt.np(alloc.dtype)
                out_avals.append(jax.core.ShapedArray(shape, dtype))
                zero_outs.append(np.zeros(shape, dtype))
        self.in_names = in_names
        n_params = len(in_names)
        n_outs = len(out_avals)
        in_names_full = list(in_names) + out_names
        if partition_name is not None:
            in_names_full.append(partition_name)

        devices = jax.devices()[:NCORES]
        mesh = Mesh(np.asarray(devices), ("core",))
        self.sharding = NamedSharding(mesh, PartitionSpec("core"))

        def _body(*args):
            operands = list(args)
            if partition_name is not None:
                operands.append(partition_id_tensor())
            outs = _bass_exec_p.bind(
                *operands,
                out_avals=tuple(out_avals),
                in_names=tuple(in_names_full),
                out_names=tuple(out_names),
                lowering_input_output_aliases=(),
                sim_require_finite=True,
                sim_require_nnan=True,
                nc=nc,
            )
            return tuple(outs)

        in_specs = (PartitionSpec("core"),) * (n_params + n_outs)
        out_specs = (PartitionSpec("core"),) * n_outs
        # No donation: the kernel writes every output element, so the
        # pre-zeroed output operands can live on-device permanently.
        self.fn = jax.jit(
            shard_map(_body, mesh=mesh, in_specs=in_specs,
                      out_specs=out_specs, check_rep=False),
            keep_unused=True)
        self.dz = [jax.device_put(
            np.zeros((NCORES * z.shape[0], *z.shape[1:]), z.dtype),
            self.sharding) for z in zero_outs]

    def run(self, concat_map):
        args = [concat_map[name] for name in self.in_names]
        out = self.fn(*args, *self.dz)
        return [np.asarray(o) for o in out]


def _assemble(o):
    """o: (NCORES*UNITS, P, F) uint8 tiles -> full (N, N) symmetrized fp32."""
    out = np.zeros((N, N), dtype=np.float32)
    for idx, (bi, col0) in enumerate(_UNITS):
        r0 = bi * P
        tile = _DEQ_LUT[o[idx]]
        c_lo, c_hi = col0, col0 + F
        if c_lo <= r0 < c_hi:
            # diagonal block inside this tile: keep strictly-upper, mirror;
            # cols < r0 are below-diagonal (wrong-side values) -> skip.
            d0 = r0 - c_lo
            dblk = tile[:, d0:d0 + P] * _TRIU_MASK_P
            out[r0:r0 + P, r0:r0 + P] = dblk
            out[r0:r0 + P, r0:r0 + P] += dblk.T
            if d0 + P < F:
                post = tile[:, d0 + P:]
                out[r0:r0 + P, r0 + P:c_hi] = post
                out[r0 + P:c_hi, r0:r0 + P] = post.T
        else:
            out[r0:r0 + P, c_lo:c_hi] = tile
            out[c_lo:c_hi, r0:r0 + P] = tile.T
    return out


def kernel(loop_embeddings, W1, b1, W2, b2):
    concat, pos_mask = _prep_inputs(loop_embeddings, W1, b1, W2, b2)

    if pos_mask not in _cache:
        nc = _build_module(pos_mask)
        _cache[pos_mask] = _Executor(nc)
    ex = _cache[pos_mask]

    outs = ex.run(concat)
    o = outs[0].reshape(NCORES * UNITS_PER_CORE, P, F)
    return _assemble(o)


# revision 10
# speedup vs baseline: 5.3908x; 1.2130x over previous
"""Trainium2 Bass kernel for LoopConnectivityDecoder.

Math: out[i,j] (i<j) = sigmoid( sum_k W2[k] * relu(a'[i,k] + b'[k,j]) + b2 ),
symmetrized, zero diagonal; a' = X@W1[:,:32].T + b1, b' = (X@W1[:,32:].T).T.

The axon tunnel dominates cost (~70-80ms fixed round-trip, ~50-70MB/s), so
the design minimizes per-call host<->device traffic and keeps a persistent
jitted executable:

- Inputs per core are raw-ish and tiny (~125KB fp16): X.T column-gathers for
  the core's units (XA/XB), slot-permuted w2-scaled W1 halves (Wa/Wb), biases.
- The device computes az[s,i] = w2_s*(X@W1a.T + b1)[i,s] and
  bz[s,j] = w2_s*(X@W1b.T)[j,s] itself with K=32 matmuls into PSUM, drained
  to fp16 SBUF (slot-permuted so sign groups are contiguous).
- Upper triangle covered by 24 uniform (128 x 512) units, 3 per core. Per
  k-slot, z[i,j] = az[s,i] + bz[s,j] via two K=1 PE matmuls accumulating in
  PSUM: (az_row)^T @ ones + ones^T @ bz_row.
- k's are sign-grouped, chunked by 4: ScalarE drains each (128,4,512) PSUM
  tile with fused relu (scale=+/-1), VectorE runs the signed accumulate
  chain, then sigmoid(+b2) and a uint8 quantization (x255) per unit tile.
- Output is uint8 (sigmoid in [0,1]; quantization error ~0.2% << 2e-2
  tolerance), 196KB/core. Host dequantizes + scatters + mirrors.
"""

import numpy as np

N = 1536
EMB = 32
H = 64
P = 128          # partition tile (rows per unit)
F = 512          # free-dim tile (cols per unit)
NCORES = 8
NBLK = N // P    # 12 row blocks
UNITS_PER_CORE = 3
CH = 4           # k's per chunk (PSUM tile = CH banks)

_cache = {}


def _unit_list():
    """24 (row_block, col0) units covering the upper-triangle staircase."""
    units = []
    for bi in range(NBLK):
        cols = N - P * bi
        nch = -(-cols // F)
        for t in range(nch):
            col0 = min(P * bi + F * t, N - F)
            units.append((bi, col0))
    assert len(units) == NCORES * UNITS_PER_CORE
    return units


def _slot_list(pos_mask, ch=CH):
    """Sign-grouped, zero-padded slot list.

    Returns (slots, chunk_signs): slots[i] is a k index or None (zero pad);
    chunk_signs[c] is +1/-1 for slots[ch*c : ch*(c+1)]."""
    pos = [k for k in range(H) if pos_mask[k]]
    neg = [k for k in range(H) if not pos_mask[k]]
    slots, signs = [], []
    for grp, sgn in ((pos, 1.0), (neg, -1.0)):
        if not grp:
            continue
        pad = (-len(grp)) % ch
        g = [None] * pad + grp
        slots += g
        signs += [sgn] * (len(g) // ch)
    assert len(slots) % ch == 0
    return slots, signs


def _static_maps():
    """Cached static gather indices for the unit layout."""
    units = _unit_list()
    acols = np.empty((NCORES, UNITS_PER_CORE * P), dtype=np.int64)
    bcols = np.empty((NCORES, UNITS_PER_CORE * F), dtype=np.int64)
    for core in range(NCORES):
        for u in range(UNITS_PER_CORE):
            bi, col0 = units[core * UNITS_PER_CORE + u]
            acols[core, u * P:(u + 1) * P] = np.arange(bi * P, (bi + 1) * P)
            bcols[core, u * F:(u + 1) * F] = np.arange(col0, col0 + F)
    return units, acols, bcols


_UNITS, _ACOLS, _BCOLS = _static_maps()
_TRIU_MASK_P = np.triu(np.ones((P, P), dtype=bool), k=1)
_DEQ_LUT = (np.arange(256, dtype=np.float32) / 255.0)


def _build_module(pos_mask, ch=CH):
    """Build + compile the Bass module. pos_mask: tuple of 64 bools."""
    from contextlib import ExitStack
    import concourse.tile as tile
    from concourse import bacc, mybir

    slots, signs = _slot_list(pos_mask, ch)
    S = len(slots)
    NCH = S // ch
    f16 = mybir.dt.float16
    f32 = mybir.dt.float32

    nc = bacc.Bacc("TRN2", target_bir_lowering=False, debug=False,
                   num_devices=NCORES)
    XA_d = nc.dram_tensor("XAg", [EMB, UNITS_PER_CORE * P], f16,
                          kind="ExternalInput")
    XB_d = nc.dram_tensor("XBg", [EMB, UNITS_PER_CORE * F], f16,
                          kind="ExternalInput")
    Wa_d = nc.dram_tensor("Wag", [EMB, S], f16, kind="ExternalInput")
    Wb_d = nc.dram_tensor("Wbg", [EMB, S], f16, kind="ExternalInput")
    ba_d = nc.dram_tensor("bag", [S, 1], f32, kind="ExternalInput")
    b2_d = nc.dram_tensor("b2c", [P, 1], f32, kind="ExternalInput")
    out_d = nc.dram_tensor("out", [UNITS_PER_CORE, P, F], mybir.dt.uint8,
                           kind="ExternalOutput")

    with tile.TileContext(nc) as tc, ExitStack() as ctx:
        const = ctx.enter_context(tc.tile_pool(name="const", bufs=1))
        bfp = ctx.enter_context(tc.tile_pool(name="bfp", bufs=1))
        stg = ctx.enter_context(tc.tile_pool(name="stg", bufs=2))
        accp = ctx.enter_context(tc.tile_pool(name="accp", bufs=2))
        outp = ctx.enter_context(tc.tile_pool(name="outp", bufs=2))
        psprep = ctx.enter_context(tc.tile_pool(name="psprep", bufs=1,
                                                space="PSUM"))
        psum = ctx.enter_context(tc.tile_pool(name="psum", bufs=1,
                                              space="PSUM"))

        XA_t = const.tile([EMB, UNITS_PER_CORE * P], f16)
        XB_t = const.tile([EMB, UNITS_PER_CORE * F], f16)
        Wa_t = const.tile([EMB, S], f16)
        Wb_t = const.tile([EMB, S], f16)
        ba_t = const.tile([S, 1], f32)
        b2_t = const.tile([P, 1], f32)
        nc.sync.dma_start(XA_t[:], XA_d[:])
        nc.sync.dma_start(XB_t[:], XB_d[:])
        nc.sync.dma_start(Wa_t[:], Wa_d[:])
        nc.sync.dma_start(Wb_t[:], Wb_d[:])
        nc.sync.dma_start(ba_t[:], ba_d[:])
        nc.sync.dma_start(b2_t[:], b2_d[:])

        onesP = const.tile([1, P], f16)
        onesF = const.tile([1, F], f16)
        nc.vector.memset(onesP[:], 1.0)
        nc.vector.memset(onesF[:], 1.0)

        # on-device operand prep: az/bz for all 3 units, slot-permuted
        psA = psprep.tile([S, UNITS_PER_CORE, P], f32)
        psB = psprep.tile([S, UNITS_PER_CORE, F], f32)
        for u in range(UNITS_PER_CORE):
            nc.tensor.matmul(psA[:, u], Wa_t[:], XA_t[:, u * P:(u + 1) * P],
                             start=True, stop=True)
            nc.tensor.matmul(psB[:, u], Wb_t[:], XB_t[:, u * F:(u + 1) * F],
                             start=True, stop=True)
        azsb = const.tile([S, UNITS_PER_CORE, P], f16)
        bzsb = const.tile([S, UNITS_PER_CORE, F], f16)
        nc.scalar.activation(azsb[:], psA[:],
                             mybir.ActivationFunctionType.Identity,
                             bias=ba_t[:, 0:1], scale=1.0)
        nc.scalar.activation(bzsb[:], psB[:],
                             mybir.ActivationFunctionType.Identity)

        # PE matmul operands must start at partition 0/32/64, so flatten the
        # per-slot rows onto partition 0 (slots along the free dim).
        af = const.tile([1, UNITS_PER_CORE, S, P], f16)
        for u in range(UNITS_PER_CORE):
            nc.sync.dma_start(af[0:1, u], azsb[:, u, :])

        for u in range(UNITS_PER_CORE):
            bf = bfp.tile([1, S, F], f16, tag="bf")
            nc.sync.dma_start(bf[0:1], bzsb[:, u, :])
            accD = None
            for c in range(NCH):
                sgn = signs[c]
                y = psum.tile([P, ch, F], f32, tag="y")
                for q in range(ch):
                    s = c * ch + q
                    nc.tensor.matmul(y[:, q], af[0:1, u, s, :],
                                     onesF[0:1, :], start=True, stop=False)
                    nc.tensor.matmul(y[:, q], onesP[0:1, :],
                                     bf[0:1, s, :],
                                     start=False, stop=True)
                t4 = stg.tile([P, ch, F], f32, tag="t4")
                nc.scalar.activation(t4[:], y[:],
                                     mybir.ActivationFunctionType.Relu,
                                     scale=float(sgn))
                newacc = accp.tile([P, ch, F], f32, tag="accD")
                if accD is None:
                    nc.vector.tensor_scalar(newacc[:], t4[:], float(sgn),
                                            None, mybir.AluOpType.mult)
                else:
                    nc.vector.scalar_tensor_tensor(
                        newacc[:], t4[:], float(sgn), accD[:],
                        mybir.AluOpType.mult, mybir.AluOpType.add)
                accD = newacc

            # fold ch slices -> logit, sigmoid, quantize, store
            acc, w = accD, ch
            while w > 1:
                half = w // 2
                nxt = outp.tile([P, half, F], f32, tag=f"fold{half}")
                nc.vector.tensor_tensor(nxt[:], acc[:, 0:half],
                                        acc[:, half:2 * half],
                                        mybir.AluOpType.add)
                acc, w = nxt, half
            s_t = outp.tile([P, F], f32, tag="s")
            nc.scalar.activation(s_t[:], acc[:, 0],
                                 mybir.ActivationFunctionType.Sigmoid,
                                 bias=b2_t[:, 0:1], scale=1.0)
            q_t = outp.tile([P, F], mybir.dt.uint8, tag="q")
            nc.vector.tensor_scalar(q_t[:], s_t[:], 255.0, None,
                                    mybir.AluOpType.mult)
            nc.sync.dma_start(out_d[u], q_t[:])

    nc.compile()
    return nc


def _prep_inputs(loop_embeddings, W1, b1, W2, b2):
    """Vectorized, tiny per-core input build. Returns (concat dict, pos_mask)."""
    X = np.asarray(loop_embeddings, dtype=np.float32)
    W1 = np.asarray(W1, dtype=np.float32)
    b1 = np.asarray(b1, dtype=np.float32)
    W2 = np.asarray(W2, dtype=np.float32)
    b2 = np.asarray(b2, dtype=np.float32)
    w2 = W2[0]

    pos_mask = tuple(bool(v) for v in (w2 >= 0))
    slots, _ = _slot_list(pos_mask)
    S = len(slots)
    kmap = np.array([0 if k is None else k for k in slots], dtype=np.int64)
    kvalid = np.array([k is not None for k in slots], dtype=bool)

    Wa = (w2[None, :] * W1[:, :EMB].T)[:, kmap].astype(np.float16)
    Wb = (w2[None, :] * W1[:, EMB:].T)[:, kmap].astype(np.float16)
    Wa[:, ~kvalid] = 0
    Wb[:, ~kvalid] = 0
    ba = (w2 * b1)[kmap].astype(np.float32)
    ba[~kvalid] = 0

    XT16 = X.T.astype(np.float16)                      # (EMB, N)
    XA = XT16[:, _ACOLS].transpose(1, 0, 2)            # (NCORES, EMB, 3P)
    XB = XT16[:, _BCOLS].transpose(1, 0, 2)            # (NCORES, EMB, 3F)

    concat = {
        "XAg": np.ascontiguousarray(XA).reshape(NCORES * EMB, -1),
        "XBg": np.ascontiguousarray(XB).reshape(NCORES * EMB, -1),
        "Wag": np.tile(Wa, (NCORES, 1)),
        "Wbg": np.tile(Wb, (NCORES, 1)),
        "bag": np.tile(ba[:, None], (NCORES, 1)),
        "b2c": np.full((NCORES * P, 1), b2[0], dtype=np.float32),
    }
    return concat, pos_mask


class _Executor:
    """Persistent jitted shard_map executable for a compiled Bass module."""

    def __init__(self, nc):
        import jax
        from jax.sharding import Mesh, PartitionSpec, NamedSharding
        from jax.experimental.shard_map import shard_map
        from concourse import mybir
        from concourse.bass2jax import (_bass_exec_p, install_neuronx_cc_hook,
                                        partition_id_tensor)

        install_neuronx_cc_hook()
        self.nc = nc
        partition_name = (nc.partition_id_tensor.name
                          if nc.partition_id_tensor else None)
        in_names, out_names, out_avals, zero_outs = [], [], [], []
        for alloc in nc.m.functions[0].allocations:
            if not isinstance(alloc, mybir.MemoryLocationSet):
                continue
            name = alloc.memorylocations[0].name
            if alloc.kind == "ExternalInput":
                if name != partition_name:
                    in_names.append(name)
            elif alloc.kind == "ExternalOutput":
                out_names.append(name)
                shape = tuple(alloc.tensor_shape)
                dtype = mybir.dt.np(alloc.dtype)
                out_avals.append(jax.core.ShapedArray(shape, dtype))
                zero_outs.append(np.zeros(shape, dtype))
        self.in_names = in_names
        n_params = len(in_names)
        n_outs = len(out_avals)
        in_names_full = list(in_names) + out_names
        if partition_name is not None:
            in_names_full.append(partition_name)

        devices = jax.devices()[:NCORES]
        mesh = Mesh(np.asarray(devices), ("core",))
        self.sharding = NamedSharding(mesh, PartitionSpec("core"))

        def _body(*args):
            operands = list(args)
            if partition_name is not None:
                operands.append(partition_id_tensor())
            outs = _bass_exec_p.bind(
                *operands,
                out_avals=tuple(out_avals),
                in_names=tuple(in_names_full),
                out_names=tuple(out_names),
                lowering_input_output_aliases=(),
                sim_require_finite=True,
                sim_require_nnan=True,
                nc=nc,
            )
            return tuple(outs)

        in_specs = (PartitionSpec("core"),) * (n_params + n_outs)
        out_specs = (PartitionSpec("core"),) * n_outs
        # No donation: the kernel writes every output element, so the
        # pre-zeroed output operands can live on-device permanently.
        self.fn = jax.jit(
            shard_map(_body, mesh=mesh, in_specs=in_specs,
                      out_specs=out_specs, check_rep=False),
            keep_unused=True)
        self.dz = [jax.device_put(
            np.zeros((NCORES * z.shape[0], *z.shape[1:]), z.dtype),
            self.sharding) for z in zero_outs]

    def run(self, concat_map):
        args = [concat_map[name] for name in self.in_names]
        out = self.fn(*args, *self.dz)
        return [np.asarray(o) for o in out]


def _scatter_unit(out, unit, tile):
    """Scatter one dequantized (P, F) tile + its mirror into out."""
    bi, col0 = unit
    r0 = bi * P
    c_lo, c_hi = col0, col0 + F
    if c_lo <= r0 < c_hi:
        # diagonal block inside this tile: keep strictly-upper, mirror;
        # cols < r0 are below-diagonal (wrong-side values) -> skip.
        d0 = r0 - c_lo
        dblk = tile[:, d0:d0 + P] * _TRIU_MASK_P
        out[r0:r0 + P, r0:r0 + P] = dblk
        out[r0:r0 + P, r0:r0 + P] += dblk.T
        if d0 + P < F:
            post = tile[:, d0 + P:]
            out[r0:r0 + P, r0 + P:c_hi] = post
            out[r0 + P:c_hi, r0:r0 + P] = post.T
    else:
        out[r0:r0 + P, c_lo:c_hi] = tile
        out[c_lo:c_hi, r0:r0 + P] = tile.T


def _assemble(o):
    """o: (NCORES*UNITS, P, F) uint8 tiles -> full (N, N) symmetrized fp32."""
    out = np.zeros((N, N), dtype=np.float32)
    for idx, unit in enumerate(_UNITS):
        _scatter_unit(out, unit, _DEQ_LUT[o[idx]])
    return out


_POOL = None


def kernel(loop_embeddings, W1, b1, W2, b2):
    global _POOL
    concat, pos_mask = _prep_inputs(loop_embeddings, W1, b1, W2, b2)

    if pos_mask not in _cache:
        nc = _build_module(pos_mask)
        _cache[pos_mask] = _Executor(nc)
    ex = _cache[pos_mask]

    args = [concat[name] for name in ex.in_names]
    out = ex.fn(*args, *ex.dz)[0]

    # Fetch per-core shards in parallel threads (the transfer releases the
    # GIL) and assemble each as it lands, hiding host scatter in the fetch.
    result = np.zeros((N, N), dtype=np.float32)
    shards = out.addressable_shards

    def work(item):
        pos, sh = item
        start = sh.index[0].start if sh.index else None
        core = pos if start is None else start // UNITS_PER_CORE
        tiles = _DEQ_LUT[np.asarray(sh.data)]
        for u in range(UNITS_PER_CORE):
            _scatter_unit(result, _UNITS[core * UNITS_PER_CORE + u], tiles[u])

    if _POOL is None:
        from concurrent.futures import ThreadPoolExecutor
        _POOL = ThreadPoolExecutor(NCORES)
    list(_POOL.map(work, enumerate(shards)))
    return result


# revision 16
# speedup vs baseline: 5.6848x; 1.0545x over previous
"""Trainium2 Bass kernel for LoopConnectivityDecoder.

Math: out[i,j] (i<j) = sigmoid( sum_k W2[k] * relu(a'[i,k] + b'[k,j]) + b2 ),
symmetrized, zero diagonal; a' = X@W1[:,:32].T + b1, b' = (X@W1[:,32:].T).T.

The axon tunnel dominates cost (~70-80ms fixed round-trip, ~50-70MB/s), so
the design minimizes per-call host<->device traffic and keeps a persistent
jitted executable:

- Inputs per core are raw-ish and tiny (~125KB fp16): X.T column-gathers for
  the core's units (XA/XB), slot-permuted w2-scaled W1 halves (Wa/Wb), biases.
- The device computes az[s,i] = w2_s*(X@W1a.T + b1)[i,s] and
  bz[s,j] = w2_s*(X@W1b.T)[j,s] itself with K=32 matmuls into PSUM, drained
  to fp16 SBUF (slot-permuted so sign groups are contiguous).
- Upper triangle covered by 24 uniform (128 x 512) units, 3 per core. Per
  k-slot, z[i,j] = az[s,i] + bz[s,j] via two K=1 PE matmuls accumulating in
  PSUM: (az_row)^T @ ones + ones^T @ bz_row.
- k's are sign-grouped, chunked by 4: ScalarE drains each (128,4,512) PSUM
  tile with fused relu (scale=+/-1), VectorE runs the signed accumulate
  chain, then sigmoid(+b2) and a uint8 quantization (x255) per unit tile.
- Output is uint8 (sigmoid in [0,1]; quantization error ~0.2% << 2e-2
  tolerance), 196KB/core. Host dequantizes + scatters + mirrors.
"""

import numpy as np

N = 1536
EMB = 32
H = 64
P = 128          # partition tile (rows per unit)
F = 512          # free-dim tile (cols per unit)
NCORES = 8
NBLK = N // P    # 12 row blocks
UNITS_PER_CORE = 3
CH = 4           # k's per chunk (PSUM tile = CH banks)

_cache = {}


def _unit_list():
    """24 (row_block, col0) units covering the upper-triangle staircase,
    ordered so that each core's unit0 and unit2 share a column range
    (XSLOT pattern (0,1,0)), letting XB carry 2 column blocks, not 3."""
    units = [
        (0, 1024), (0, 0),   (1, 1024),
        (2, 1024), (1, 128), (3, 1024),
        (4, 1024), (2, 256), (5, 1024),
        (6, 1024), (3, 384), (7, 1024),
        (8, 1024), (2, 768), (9, 1024),
        (10, 1024), (3, 896), (11, 1024),
        (0, 512), (6, 768), (4, 512),
        (1, 640), (7, 896), (5, 640),
    ]
    # sanity: covers the staircase exactly once
    ref = []
    for bi in range(NBLK):
        cols = N - P * bi
        nch = -(-cols // F)
        for t in range(nch):
            ref.append((bi, min(P * bi + F * t, N - F)))
    assert sorted(units) == sorted(ref)
    for core in range(NCORES):
        assert units[core * 3][1] == units[core * 3 + 2][1]
    return units


XSLOT = (0, 1, 0)        # per-unit column-block slice into XB
NXB = 2                  # distinct column blocks shipped per core


def _slot_list(pos_mask, ch=CH):
    """Sign-grouped, zero-padded slot list.

    Returns (slots, chunk_signs): slots[i] is a k index or None (zero pad);
    chunk_signs[c] is +1/-1 for slots[ch*c : ch*(c+1)]."""
    pos = [k for k in range(H) if pos_mask[k]]
    neg = [k for k in range(H) if not pos_mask[k]]
    slots, signs = [], []
    for grp, sgn in ((pos, 1.0), (neg, -1.0)):
        if not grp:
            continue
        pad = (-len(grp)) % ch
        g = [None] * pad + grp
        slots += g
        signs += [sgn] * (len(g) // ch)
    assert len(slots) % ch == 0
    return slots, signs


def _static_maps():
    """Cached static gather indices for the unit layout."""
    units = _unit_list()
    acols = np.empty((NCORES, UNITS_PER_CORE * P), dtype=np.int64)
    bcols = np.empty((NCORES, NXB * F), dtype=np.int64)
    for core in range(NCORES):
        for u in range(UNITS_PER_CORE):
            bi, col0 = units[core * UNITS_PER_CORE + u]
            acols[core, u * P:(u + 1) * P] = np.arange(bi * P, (bi + 1) * P)
            x = XSLOT[u]
            bcols[core, x * F:(x + 1) * F] = np.arange(col0, col0 + F)
    return units, acols, bcols


_UNITS, _ACOLS, _BCOLS = _static_maps()
_TRIU_MASK_P = np.triu(np.ones((P, P), dtype=bool), k=1)
_DEQ_LUT = (np.arange(256, dtype=np.float32) / 255.0)


def _build_module(pos_mask, ch=CH):
    """Build + compile the Bass module. pos_mask: tuple of 64 bools."""
    from contextlib import ExitStack
    import concourse.tile as tile
    from concourse import bacc, mybir

    slots, signs = _slot_list(pos_mask, ch)
    S = len(slots)
    NCH = S // ch
    f16 = mybir.dt.float16
    f32 = mybir.dt.float32

    nc = bacc.Bacc("TRN2", target_bir_lowering=False, debug=False,
                   num_devices=NCORES)
    XA_d = nc.dram_tensor("XAg", [EMB, UNITS_PER_CORE * P], f16,
                          kind="ExternalInput")
    XB_d = nc.dram_tensor("XBg", [EMB, NXB * F], f16,
                          kind="ExternalInput")
    Wa_d = nc.dram_tensor("Wag", [EMB, S], f16, kind="ExternalInput")
    Wb_d = nc.dram_tensor("Wbg", [EMB, S], f16, kind="ExternalInput")
    ba_d = nc.dram_tensor("bag", [S, 1], f32, kind="ExternalInput")
    b2_d = nc.dram_tensor("b2c", [P, 1], f32, kind="ExternalInput")
    out_d = nc.dram_tensor("out", [UNITS_PER_CORE, P, F], mybir.dt.uint8,
                           kind="ExternalOutput")

    with tile.TileContext(nc) as tc, ExitStack() as ctx:
        const = ctx.enter_context(tc.tile_pool(name="const", bufs=1))
        bfp = ctx.enter_context(tc.tile_pool(name="bfp", bufs=1))
        stg = ctx.enter_context(tc.tile_pool(name="stg", bufs=2))
        accp = ctx.enter_context(tc.tile_pool(name="accp", bufs=2))
        outp = ctx.enter_context(tc.tile_pool(name="outp", bufs=2))
        psprep = ctx.enter_context(tc.tile_pool(name="psprep", bufs=1,
                                                space="PSUM"))
        psum = ctx.enter_context(tc.tile_pool(name="psum", bufs=1,
                                              space="PSUM"))

        XA_t = const.tile([EMB, UNITS_PER_CORE * P], f16)
        XB_t = const.tile([EMB, NXB * F], f16)
        Wa_t = const.tile([EMB, S], f16)
        Wb_t = const.tile([EMB, S], f16)
        ba_t = const.tile([S, 1], f32)
        b2_t = const.tile([P, 1], f32)
        nc.sync.dma_start(XA_t[:], XA_d[:])
        nc.sync.dma_start(XB_t[:], XB_d[:])
        nc.sync.dma_start(Wa_t[:], Wa_d[:])
        nc.sync.dma_start(Wb_t[:], Wb_d[:])
        nc.sync.dma_start(ba_t[:], ba_d[:])
        nc.sync.dma_start(b2_t[:], b2_d[:])

        onesP = const.tile([1, P], f16)
        onesF = const.tile([1, F], f16)
        nc.vector.memset(onesP[:], 1.0)
        nc.vector.memset(onesF[:], 1.0)

        # on-device operand prep: az/bz for all 3 units, slot-permuted
        psA = psprep.tile([S, UNITS_PER_CORE, P], f32)
        psB = psprep.tile([S, NXB, F], f32)
        for u in range(UNITS_PER_CORE):
            nc.tensor.matmul(psA[:, u], Wa_t[:], XA_t[:, u * P:(u + 1) * P],
                             start=True, stop=True)
        for x in range(NXB):
            nc.tensor.matmul(psB[:, x], Wb_t[:], XB_t[:, x * F:(x + 1) * F],
                             start=True, stop=True)
        azsb = const.tile([S, UNITS_PER_CORE, P], f16)
        bzsb = const.tile([S, NXB, F], f16)
        nc.scalar.activation(azsb[:], psA[:],
                             mybir.ActivationFunctionType.Identity,
                             bias=ba_t[:, 0:1], scale=1.0)
        nc.scalar.activation(bzsb[:], psB[:],
                             mybir.ActivationFunctionType.Identity)

        # PE matmul operands must start at partition 0/32/64, so flatten the
        # per-slot rows onto partition 0 (slots along the free dim).
        af = const.tile([1, UNITS_PER_CORE, S, P], f16)
        for u in range(UNITS_PER_CORE):
            nc.sync.dma_start(af[0:1, u], azsb[:, u, :])

        for u in range(UNITS_PER_CORE):
            bf = bfp.tile([1, S, F], f16, tag="bf")
            nc.sync.dma_start(bf[0:1], bzsb[:, XSLOT[u], :])
            accD = None
            for c in range(NCH):
                sgn = signs[c]
                y = psum.tile([P, ch, F], f32, tag="y")
                for q in range(ch):
                    s = c * ch + q
                    nc.tensor.matmul(y[:, q], af[0:1, u, s, :],
                                     onesF[0:1, :], start=True, stop=False)
                    nc.tensor.matmul(y[:, q], onesP[0:1, :],
                                     bf[0:1, s, :],
                                     start=False, stop=True)
                t4 = stg.tile([P, ch, F], f32, tag="t4")
                nc.scalar.activation(t4[:], y[:],
                                     mybir.ActivationFunctionType.Relu,
                                     scale=float(sgn))
                newacc = accp.tile([P, ch, F], f32, tag="accD")
                if accD is None:
                    nc.vector.tensor_scalar(newacc[:], t4[:], float(sgn),
                                            None, mybir.AluOpType.mult)
                else:
                    nc.vector.scalar_tensor_tensor(
                        newacc[:], t4[:], float(sgn), accD[:],
                        mybir.AluOpType.mult, mybir.AluOpType.add)
                accD = newacc

            # fold ch slices -> logit, sigmoid, quantize, store
            acc, w = accD, ch
            while w > 1:
                half = w // 2
                nxt = outp.tile([P, half, F], f32, tag=f"fold{half}")
                nc.vector.tensor_tensor(nxt[:], acc[:, 0:half],
                                        acc[:, half:2 * half],
                                        mybir.AluOpType.add)
                acc, w = nxt, half
            s_t = outp.tile([P, F], f32, tag="s")
            nc.scalar.activation(s_t[:], acc[:, 0],
                                 mybir.ActivationFunctionType.Sigmoid,
                                 bias=b2_t[:, 0:1], scale=1.0)
            q_t = outp.tile([P, F], mybir.dt.uint8, tag="q")
            nc.vector.tensor_scalar(q_t[:], s_t[:], 255.0, None,
                                    mybir.AluOpType.mult)
            nc.sync.dma_start(out_d[u], q_t[:])

    nc.compile()
    return nc


def _prep_inputs(loop_embeddings, W1, b1, W2, b2):
    """Vectorized, tiny per-core input build. Returns (concat dict, pos_mask)."""
    X = np.asarray(loop_embeddings, dtype=np.float32)
    W1 = np.asarray(W1, dtype=np.float32)
    b1 = np.asarray(b1, dtype=np.float32)
    W2 = np.asarray(W2, dtype=np.float32)
    b2 = np.asarray(b2, dtype=np.float32)
    w2 = W2[0]

    pos_mask = tuple(bool(v) for v in (w2 >= 0))
    slots, _ = _slot_list(pos_mask)
    S = len(slots)
    kmap = np.array([0 if k is None else k for k in slots], dtype=np.int64)
    kvalid = np.array([k is not None for k in slots], dtype=bool)

    Wa = (w2[None, :] * W1[:, :EMB].T)[:, kmap].astype(np.float16)
    Wb = (w2[None, :] * W1[:, EMB:].T)[:, kmap].astype(np.float16)
    Wa[:, ~kvalid] = 0
    Wb[:, ~kvalid] = 0
    ba = (w2 * b1)[kmap].astype(np.float32)
    ba[~kvalid] = 0

    XT16 = X.T.astype(np.float16)                      # (EMB, N)
    XA = XT16[:, _ACOLS].transpose(1, 0, 2)            # (NCORES, EMB, 3P)
    XB = XT16[:, _BCOLS].transpose(1, 0, 2)            # (NCORES, EMB, 3F)

    concat = {
        "XAg": np.ascontiguousarray(XA).reshape(NCORES * EMB, -1),
        "XBg": np.ascontiguousarray(XB).reshape(NCORES * EMB, -1),
        "Wag": np.tile(Wa, (NCORES, 1)),
        "Wbg": np.tile(Wb, (NCORES, 1)),
        "bag": np.tile(ba[:, None], (NCORES, 1)),
        "b2c": np.full((NCORES * P, 1), b2[0], dtype=np.float32),
    }
    return concat, pos_mask


class _Executor:
    """Persistent jitted shard_map executable for a compiled Bass module."""

    def __init__(self, nc):
        import jax
        from jax.sharding import Mesh, PartitionSpec, NamedSharding
        from jax.experimental.shard_map import shard_map
        from concourse import mybir
        from concourse.bass2jax import (_bass_exec_p, install_neuronx_cc_hook,
                                        partition_id_tensor)

        install_neuronx_cc_hook()
        self.nc = nc
        partition_name = (nc.partition_id_tensor.name
                          if nc.partition_id_tensor else None)
        in_names, out_names, out_avals, zero_outs = [], [], [], []
        for alloc in nc.m.functions[0].allocations:
            if not isinstance(alloc, mybir.MemoryLocationSet):
                continue
            name = alloc.memorylocations[0].name
            if alloc.kind == "ExternalInput":
                if name != partition_name:
                    in_names.append(name)
            elif alloc.kind == "ExternalOutput":
                out_names.append(name)
                shape = tuple(alloc.tensor_shape)
                dtype = mybir.dt.np(alloc.dtype)
                out_avals.append(jax.core.ShapedArray(shape, dtype))
                zero_outs.append(np.zeros(shape, dtype))
        self.in_names = in_names
        n_params = len(in_names)
        n_outs = len(out_avals)
        in_names_full = list(in_names) + out_names
        if partition_name is not None:
            in_names_full.append(partition_name)

        devices = jax.devices()[:NCORES]
        mesh = Mesh(np.asarray(devices), ("core",))
        self.sharding = NamedSharding(mesh, PartitionSpec("core"))

        def _body(*args):
            operands = list(args)
            if partition_name is not None:
                operands.append(partition_id_tensor())
            outs = _bass_exec_p.bind(
                *operands,
                out_avals=tuple(out_avals),
                in_names=tuple(in_names_full),
                out_names=tuple(out_names),
                lowering_input_output_aliases=(),
                sim_require_finite=True,
                sim_require_nnan=True,
                nc=nc,
            )
            return tuple(outs)

        in_specs = (PartitionSpec("core"),) * (n_params + n_outs)
        out_specs = (PartitionSpec("core"),) * n_outs
        # No donation: the kernel writes every output element, so the
        # pre-zeroed output operands can live on-device permanently.
        self.fn = jax.jit(
            shard_map(_body, mesh=mesh, in_specs=in_specs,
                      out_specs=out_specs, check_rep=False),
            keep_unused=True)
        self.dz = [jax.device_put(
            np.zeros((NCORES * z.shape[0], *z.shape[1:]), z.dtype),
            self.sharding) for z in zero_outs]

    def run(self, concat_map):
        args = [concat_map[name] for name in self.in_names]
        out = self.fn(*args, *self.dz)
        return [np.asarray(o) for o in out]


def _scatter_unit(out, unit, tile):
    """Scatter one dequantized (P, F) tile + its mirror into out."""
    bi, col0 = unit
    r0 = bi * P
    c_lo, c_hi = col0, col0 + F
    if c_lo <= r0 < c_hi:
        # diagonal block inside this tile: keep strictly-upper, mirror;
        # cols < r0 are below-diagonal (wrong-side values) -> skip.
        d0 = r0 - c_lo
        dblk = tile[:, d0:d0 + P] * _TRIU_MASK_P
        out[r0:r0 + P, r0:r0 + P] = dblk
        out[r0:r0 + P, r0:r0 + P] += dblk.T
        if d0 + P < F:
            post = tile[:, d0 + P:]
            out[r0:r0 + P, r0 + P:c_hi] = post
            out[r0 + P:c_hi, r0:r0 + P] = post.T
    else:
        out[r0:r0 + P, c_lo:c_hi] = tile
        out[c_lo:c_hi, r0:r0 + P] = tile.T


def _assemble(o):
    """o: (NCORES*UNITS, P, F) uint8 tiles -> full (N, N) symmetrized fp32."""
    out = np.zeros((N, N), dtype=np.float32)
    for idx, unit in enumerate(_UNITS):
        _scatter_unit(out, unit, _DEQ_LUT[o[idx]])
    return out


_POOL = None


def kernel(loop_embeddings, W1, b1, W2, b2):
    global _POOL
    concat, pos_mask = _prep_inputs(loop_embeddings, W1, b1, W2, b2)

    if pos_mask not in _cache:
        nc = _build_module(pos_mask)
        _cache[pos_mask] = _Executor(nc)
    ex = _cache[pos_mask]

    args = [concat[name] for name in ex.in_names]
    out = ex.fn(*args, *ex.dz)[0]

    # Fetch per-core shards in parallel threads (the transfer releases the
    # GIL) and assemble each as it lands, hiding host scatter in the fetch.
    result = np.zeros((N, N), dtype=np.float32)
    shards = out.addressable_shards

    def work(item):
        pos, sh = item
        start = sh.index[0].start if sh.index else None
        core = pos if start is None else start // UNITS_PER_CORE
        tiles = _DEQ_LUT[np.asarray(sh.data)]
        for u in range(UNITS_PER_CORE):
            _scatter_unit(result, _UNITS[core * UNITS_PER_CORE + u], tiles[u])

    if _POOL is None:
        from concurrent.futures import ThreadPoolExecutor
        _POOL = ThreadPoolExecutor(NCORES)
    list(_POOL.map(work, enumerate(shards)))
    return result


# revision 17
# speedup vs baseline: 6.0709x; 1.0679x over previous
"""Trainium2 Bass kernel for LoopConnectivityDecoder.

Math: out[i,j] (i<j) = sigmoid( sum_k W2[k] * relu(a'[i,k] + b'[k,j]) + b2 ),
symmetrized, zero diagonal; a' = X@W1[:,:32].T + b1, b' = (X@W1[:,32:].T).T.

The axon tunnel dominates cost (~70-80ms fixed round-trip, ~50-70MB/s), so
the design minimizes per-call host<->device traffic and keeps a persistent
jitted executable:

- Inputs per core are raw-ish and tiny (~97KB fp16): X.T column-gathers for
  the core's units (XA: 3 row blocks; XB: 2 column blocks -- the unit->core
  assignment is chosen so unit0/unit2 share a column range, XSLOT=(0,1,0)),
  slot-permuted w2-scaled W1 halves (Wa/Wb), biases.
- The device computes az[s,i] = w2_s*(X@W1a.T + b1)[i,s] and
  bz[s,j] = w2_s*(X@W1b.T)[j,s] itself with K=32 matmuls into PSUM, drained
  to fp16 SBUF (slot-permuted so sign groups are contiguous), then
  flattened onto partition 0 by SBUF->SBUF DMA (PE operands must start at
  partition 0/32/64).
- Upper triangle covered by 24 uniform (128 x 512) units, 3 per core. Per
  k-slot, z[i,j] = az[s,i] + bz[s,j] via two K=1 PE matmuls accumulating in
  PSUM: (az_row)^T @ ones + ones^T @ bz_row.
- k's are sign-grouped, chunked by 4: ScalarE drains each (128,4,512) PSUM
  tile with fused relu (scale=+/-1), VectorE runs the signed accumulate
  chain, then sigmoid(+b2) and a uint8 quantization (x255) per unit tile.
- Output is uint8 (sigmoid in [0,1]; quantization error ~0.2% << 2e-2
  tolerance), 196KB/core. Host fetches the 8 shards in parallel threads and
  dequantizes + scatters + mirrors each as it lands (overlapped with the
  transfer).

Measured on the staged axon setup: ~98ms/call wall (was 512ms), of which
~76ms is the irreducible per-call axon protocol latency; rel err 2.3e-3.
"""

import numpy as np

N = 1536
EMB = 32
H = 64
P = 128          # partition tile (rows per unit)
F = 512          # free-dim tile (cols per unit)
NCORES = 8
NBLK = N // P    # 12 row blocks
UNITS_PER_CORE = 3
CH = 4           # k's per chunk (PSUM tile = CH banks)

_cache = {}


def _unit_list():
    """24 (row_block, col0) units covering the upper-triangle staircase,
    ordered so that each core's unit0 and unit2 share a column range
    (XSLOT pattern (0,1,0)), letting XB carry 2 column blocks, not 3."""
    units = [
        (0, 1024), (0, 0),   (1, 1024),
        (2, 1024), (1, 128), (3, 1024),
        (4, 1024), (2, 256), (5, 1024),
        (6, 1024), (3, 384), (7, 1024),
        (8, 1024), (2, 768), (9, 1024),
        (10, 1024), (3, 896), (11, 1024),
        (0, 512), (6, 768), (4, 512),
        (1, 640), (7, 896), (5, 640),
    ]
    # sanity: covers the staircase exactly once
    ref = []
    for bi in range(NBLK):
        cols = N - P * bi
        nch = -(-cols // F)
        for t in range(nch):
            ref.append((bi, min(P * bi + F * t, N - F)))
    assert sorted(units) == sorted(ref)
    for core in range(NCORES):
        assert units[core * 3][1] == units[core * 3 + 2][1]
    return units


XSLOT = (0, 1, 0)        # per-unit column-block slice into XB
NXB = 2                  # distinct column blocks shipped per core


def _slot_list(pos_mask, ch=CH):
    """Sign-grouped, zero-padded slot list.

    Returns (slots, chunk_signs): slots[i] is a k index or None (zero pad);
    chunk_signs[c] is +1/-1 for slots[ch*c : ch*(c+1)]."""
    pos = [k for k in range(H) if pos_mask[k]]
    neg = [k for k in range(H) if not pos_mask[k]]
    slots, signs = [], []
    for grp, sgn in ((pos, 1.0), (neg, -1.0)):
        if not grp:
            continue
        pad = (-len(grp)) % ch
        g = [None] * pad + grp
        slots += g
        signs += [sgn] * (len(g) // ch)
    assert len(slots) % ch == 0
    return slots, signs


def _static_maps():
    """Cached static gather indices for the unit layout."""
    units = _unit_list()
    acols = np.empty((NCORES, UNITS_PER_CORE * P), dtype=np.int64)
    bcols = np.empty((NCORES, NXB * F), dtype=np.int64)
    for core in range(NCORES):
        for u in range(UNITS_PER_CORE):
            bi, col0 = units[core * UNITS_PER_CORE + u]
            acols[core, u * P:(u + 1) * P] = np.arange(bi * P, (bi + 1) * P)
            x = XSLOT[u]
            bcols[core, x * F:(x + 1) * F] = np.arange(col0, col0 + F)
    return units, acols, bcols


_UNITS, _ACOLS, _BCOLS = _static_maps()
_TRIU_MASK_P = np.triu(np.ones((P, P), dtype=bool), k=1)
_DEQ_LUT = (np.arange(256, dtype=np.float32) / 255.0)


def _build_module(pos_mask, ch=CH):
    """Build + compile the Bass module. pos_mask: tuple of 64 bools."""
    from contextlib import ExitStack
    import concourse.tile as tile
    from concourse import bacc, mybir

    slots, signs = _slot_list(pos_mask, ch)
    S = len(slots)
    NCH = S // ch
    f16 = mybir.dt.float16
    f32 = mybir.dt.float32

    nc = bacc.Bacc("TRN2", target_bir_lowering=False, debug=False,
                   num_devices=NCORES)
    XA_d = nc.dram_tensor("XAg", [EMB, UNITS_PER_CORE * P], f16,
                          kind="ExternalInput")
    XB_d = nc.dram_tensor("XBg", [EMB, NXB * F], f16,
                          kind="ExternalInput")
    Wa_d = nc.dram_tensor("Wag", [EMB, S], f16, kind="ExternalInput")
    Wb_d = nc.dram_tensor("Wbg", [EMB, S], f16, kind="ExternalInput")
    ba_d = nc.dram_tensor("bag", [S, 1], f32, kind="ExternalInput")
    b2_d = nc.dram_tensor("b2c", [P, 1], f32, kind="ExternalInput")
    out_d = nc.dram_tensor("out", [UNITS_PER_CORE, P, F], mybir.dt.uint8,
                           kind="ExternalOutput")

    with tile.TileContext(nc) as tc, ExitStack() as ctx:
        const = ctx.enter_context(tc.tile_pool(name="const", bufs=1))
        bfp = ctx.enter_context(tc.tile_pool(name="bfp", bufs=1))
        stg = ctx.enter_context(tc.tile_pool(name="stg", bufs=2))
        accp = ctx.enter_context(tc.tile_pool(name="accp", bufs=2))
        outp = ctx.enter_context(tc.tile_pool(name="outp", bufs=2))
        psprep = ctx.enter_context(tc.tile_pool(name="psprep", bufs=1,
                                                space="PSUM"))
        psum = ctx.enter_context(tc.tile_pool(name="psum", bufs=1,
                                              space="PSUM"))

        XA_t = const.tile([EMB, UNITS_PER_CORE * P], f16)
        XB_t = const.tile([EMB, NXB * F], f16)
        Wa_t = const.tile([EMB, S], f16)
        Wb_t = const.tile([EMB, S], f16)
        ba_t = const.tile([S, 1], f32)
        b2_t = const.tile([P, 1], f32)
        nc.sync.dma_start(XA_t[:], XA_d[:])
        nc.sync.dma_start(XB_t[:], XB_d[:])
        nc.sync.dma_start(Wa_t[:], Wa_d[:])
        nc.sync.dma_start(Wb_t[:], Wb_d[:])
        nc.sync.dma_start(ba_t[:], ba_d[:])
        nc.sync.dma_start(b2_t[:], b2_d[:])

        onesP = const.tile([1, P], f16)
        onesF = const.tile([1, F], f16)
        nc.vector.memset(onesP[:], 1.0)
        nc.vector.memset(onesF[:], 1.0)

        # on-device operand prep: az/bz for all 3 units, slot-permuted
        psA = psprep.tile([S, UNITS_PER_CORE, P], f32)
        psB = psprep.tile([S, NXB, F], f32)
        for u in range(UNITS_PER_CORE):
            nc.tensor.matmul(psA[:, u], Wa_t[:], XA_t[:, u * P:(u + 1) * P],
                             start=True, stop=True)
        for x in range(NXB):
            nc.tensor.matmul(psB[:, x], Wb_t[:], XB_t[:, x * F:(x + 1) * F],
                             start=True, stop=True)
        azsb = const.tile([S, UNITS_PER_CORE, P], f16)
        bzsb = const.tile([S, NXB, F], f16)
        nc.scalar.activation(azsb[:], psA[:],
                             mybir.ActivationFunctionType.Identity,
                             bias=ba_t[:, 0:1], scale=1.0)
        nc.scalar.activation(bzsb[:], psB[:],
                             mybir.ActivationFunctionType.Identity)

        # PE matmul operands must start at partition 0/32/64, so flatten the
        # per-slot rows onto partition 0 (slots along the free dim).
        af = const.tile([1, UNITS_PER_CORE, S, P], f16)
        for u in range(UNITS_PER_CORE):
            nc.sync.dma_start(af[0:1, u], azsb[:, u, :])

        for u in range(UNITS_PER_CORE):
            bf = bfp.tile([1, S, F], f16, tag="bf")
            nc.sync.dma_start(bf[0:1], bzsb[:, XSLOT[u], :])
            accD = None
            for c in range(NCH):
                sgn = signs[c]
                y = psum.tile([P, ch, F], f32, tag="y")
                for q in range(ch):
                    s = c * ch + q
                    nc.tensor.matmul(y[:, q], af[0:1, u, s, :],
                                     onesF[0:1, :], start=True, stop=False)
                    nc.tensor.matmul(y[:, q], onesP[0:1, :],
                                     bf[0:1, s, :],
                                     start=False, stop=True)
                t4 = stg.tile([P, ch, F], f32, tag="t4")
                nc.scalar.activation(t4[:], y[:],
                                     mybir.ActivationFunctionType.Relu,
                                     scale=float(sgn))
                newacc = accp.tile([P, ch, F], f32, tag="accD")
                if accD is None:
                    nc.vector.tensor_scalar(newacc[:], t4[:], float(sgn),
                                            None, mybir.AluOpType.mult)
                else:
                    nc.vector.scalar_tensor_tensor(
                        newacc[:], t4[:], float(sgn), accD[:],
                        mybir.AluOpType.mult, mybir.AluOpType.add)
                accD = newacc

            # fold ch slices -> logit, sigmoid, quantize, store
            acc, w = accD, ch
            while w > 1:
                half = w // 2
                nxt = outp.tile([P, half, F], f32, tag=f"fold{half}")
                nc.vector.tensor_tensor(nxt[:], acc[:, 0:half],
                                        acc[:, half:2 * half],
                                        mybir.AluOpType.add)
                acc, w = nxt, half
            s_t = outp.tile([P, F], f32, tag="s")
            nc.scalar.activation(s_t[:], acc[:, 0],
                                 mybir.ActivationFunctionType.Sigmoid,
                                 bias=b2_t[:, 0:1], scale=1.0)
            q_t = outp.tile([P, F], mybir.dt.uint8, tag="q")
            nc.vector.tensor_scalar(q_t[:], s_t[:], 255.0, None,
                                    mybir.AluOpType.mult)
            nc.sync.dma_start(out_d[u], q_t[:])

    nc.compile()
    return nc


def _prep_inputs(loop_embeddings, W1, b1, W2, b2):
    """Vectorized, tiny per-core input build. Returns (concat dict, pos_mask)."""
    X = np.asarray(loop_embeddings, dtype=np.float32)
    W1 = np.asarray(W1, dtype=np.float32)
    b1 = np.asarray(b1, dtype=np.float32)
    W2 = np.asarray(W2, dtype=np.float32)
    b2 = np.asarray(b2, dtype=np.float32)
    w2 = W2[0]

    pos_mask = tuple(bool(v) for v in (w2 >= 0))
    slots, _ = _slot_list(pos_mask)
    S = len(slots)
    kmap = np.array([0 if k is None else k for k in slots], dtype=np.int64)
    kvalid = np.array([k is not None for k in slots], dtype=bool)

    Wa = (w2[None, :] * W1[:, :EMB].T)[:, kmap].astype(np.float16)
    Wb = (w2[None, :] * W1[:, EMB:].T)[:, kmap].astype(np.float16)
    Wa[:, ~kvalid] = 0
    Wb[:, ~kvalid] = 0
    ba = (w2 * b1)[kmap].astype(np.float32)
    ba[~kvalid] = 0

    XT16 = X.T.astype(np.float16)                      # (EMB, N)
    XA = XT16[:, _ACOLS].transpose(1, 0, 2)            # (NCORES, EMB, 3P)
    XB = XT16[:, _BCOLS].transpose(1, 0, 2)            # (NCORES, EMB, 3F)

    concat = {
        "XAg": np.ascontiguousarray(XA).reshape(NCORES * EMB, -1),
        "XBg": np.ascontiguousarray(XB).reshape(NCORES * EMB, -1),
        "Wag": np.tile(Wa, (NCORES, 1)),
        "Wbg": np.tile(Wb, (NCORES, 1)),
        "bag": np.tile(ba[:, None], (NCORES, 1)),
        "b2c": np.full((NCORES * P, 1), b2[0], dtype=np.float32),
    }
    return concat, pos_mask


class _Executor:
    """Persistent jitted shard_map executable for a compiled Bass module."""

    def __init__(self, nc):
        import jax
        from jax.sharding import Mesh, PartitionSpec, NamedSharding
        from jax.experimental.shard_map import shard_map
        from concourse import mybir
        from concourse.bass2jax import (_bass_exec_p, install_neuronx_cc_hook,
                                        partition_id_tensor)

        install_neuronx_cc_hook()
        self.nc = nc
        partition_name = (nc.partition_id_tensor.name
                          if nc.partition_id_tensor else None)
        in_names, out_names, out_avals, zero_outs = [], [], [], []
        for alloc in nc.m.functions[0].allocations:
            if not isinstance(alloc, mybir.MemoryLocationSet):
                continue
            name = alloc.memorylocations[0].name
            if alloc.kind == "ExternalInput":
                if name != partition_name:
                    in_names.append(name)
            elif alloc.kind == "ExternalOutput":
                out_names.append(name)
                shape = tuple(alloc.tensor_shape)
                dtype = mybir.dt.np(alloc.dtype)
                out_avals.append(jax.core.ShapedArray(shape, dtype))
                zero_outs.append(np.zeros(shape, dtype))
        self.in_names = in_names
        n_params = len(in_names)
        n_outs = len(out_avals)
        in_names_full = list(in_names) + out_names
        if partition_name is not None:
            in_names_full.append(partition_name)

        devices = jax.devices()[:NCORES]
        mesh = Mesh(np.asarray(devices), ("core",))
        self.sharding = NamedSharding(mesh, PartitionSpec("core"))

        def _body(*args):
            operands = list(args)
            if partition_name is not None:
                operands.append(partition_id_tensor())
            outs = _bass_exec_p.bind(
                *operands,
                out_avals=tuple(out_avals),
                in_names=tuple(in_names_full),
                out_names=tuple(out_names),
                lowering_input_output_aliases=(),
                sim_require_finite=True,
                sim_require_nnan=True,
                nc=nc,
            )
            return tuple(outs)

        in_specs = (PartitionSpec("core"),) * (n_params + n_outs)
        out_specs = (PartitionSpec("core"),) * n_outs
        # No donation: the kernel writes every output element, so the
        # pre-zeroed output operands can live on-device permanently.
        self.fn = jax.jit(
            shard_map(_body, mesh=mesh, in_specs=in_specs,
                      out_specs=out_specs, check_rep=False),
            keep_unused=True)
        self.dz = [jax.device_put(
            np.zeros((NCORES * z.shape[0], *z.shape[1:]), z.dtype),
            self.sharding) for z in zero_outs]

    def run(self, concat_map):
        args = [concat_map[name] for name in self.in_names]
        out = self.fn(*args, *self.dz)
        return [np.asarray(o) for o in out]


def _scatter_unit(out, unit, tile):
    """Scatter one dequantized (P, F) tile + its mirror into out."""
    bi, col0 = unit
    r0 = bi * P
    c_lo, c_hi = col0, col0 + F
    if c_lo <= r0 < c_hi:
        # diagonal block inside this tile: keep strictly-upper, mirror;
        # cols < r0 are below-diagonal (wrong-side values) -> skip.
        d0 = r0 - c_lo
        dblk = tile[:, d0:d0 + P] * _TRIU_MASK_P
        out[r0:r0 + P, r0:r0 + P] = dblk
        out[r0:r0 + P, r0:r0 + P] += dblk.T
        if d0 + P < F:
            post = tile[:, d0 + P:]
            out[r0:r0 + P, r0 + P:c_hi] = post
            out[r0 + P:c_hi, r0:r0 + P] = post.T
    else:
        out[r0:r0 + P, c_lo:c_hi] = tile
        out[c_lo:c_hi, r0:r0 + P] = tile.T


def _assemble(o):
    """o: (NCORES*UNITS, P, F) uint8 tiles -> full (N, N) symmetrized fp32."""
    out = np.zeros((N, N), dtype=np.float32)
    for idx, unit in enumerate(_UNITS):
        _scatter_unit(out, unit, _DEQ_LUT[o[idx]])
    return out


_POOL = None


def kernel(loop_embeddings, W1, b1, W2, b2):
    global _POOL
    concat, pos_mask = _prep_inputs(loop_embeddings, W1, b1, W2, b2)

    if pos_mask not in _cache:
        nc = _build_module(pos_mask)
        _cache[pos_mask] = _Executor(nc)
    ex = _cache[pos_mask]

    args = [concat[name] for name in ex.in_names]
    out = ex.fn(*args, *ex.dz)[0]

    # Fetch per-core shards in parallel threads (the transfer releases the
    # GIL) and assemble each as it lands, hiding host scatter in the fetch.
    result = np.zeros((N, N), dtype=np.float32)
    shards = out.addressable_shards

    def work(item):
        pos, sh = item
        start = sh.index[0].start if sh.index else None
        core = pos if start is None else start // UNITS_PER_CORE
        tiles = _DEQ_LUT[np.asarray(sh.data)]
        for u in range(UNITS_PER_CORE):
            _scatter_unit(result, _UNITS[core * UNITS_PER_CORE + u], tiles[u])

    if _POOL is None:
        from concurrent.futures import ThreadPoolExecutor
        _POOL = ThreadPoolExecutor(NCORES)
    list(_POOL.map(work, enumerate(shards)))
    return result
